# revision 1
# baseline (speedup 1.0000x reference)
"""MoE layer (top-2 of 8 experts) on 8 TRN2 NeuronCores.

Strategy:
  Phase 1 (device, data-parallel): each core computes gate logits
      logitsT = gate_w.T @ x_shard.T for B/8 tokens (fp32 matmul).
  Host: softmax + top-2 + renormalized weights (the routing / sharding
      decision), build per-expert token index lists, pad to a common
      capacity C (multiple of the token block).
  Phase 2 (device, expert-parallel): core e runs its expert's FFN over
      the tokens routed to it: y = (relu(x@W1+b1)@W2 + b2) * w_token.
      bf16 matmuls, fp32 PSUM accumulation, weights SBUF-resident.
  Host: scatter-add the two scaled contributions per token.
"""

import numpy as np
import ml_dtypes

import concourse.mybir as mybir
import concourse.tile as tile
from concourse import bacc
from concourse.bass_utils import run_bass_kernel_spmd

P = 128
N_CORES = 8
CB = 256  # phase-2 token block
BF16 = mybir.dt.bfloat16
F32 = mybir.dt.float32
_bf16_np = ml_dtypes.bfloat16

_build_cache = {}


def _build_gate(D, E, T):
    """Per-core gate matmul: logitsT[E, T] = gate_w[D, E].T @ xT[D, T]."""
    nc = bacc.Bacc(None, target_bir_lowering=False)
    xT = nc.dram_tensor("xT", [D, T], F32, kind="ExternalInput")
    gw = nc.dram_tensor("gw", [D, E], F32, kind="ExternalInput")
    logitsT = nc.dram_tensor("logitsT", [E, T], F32, kind="ExternalOutput")
    DO = D // P
    NT = 512
    xT_r = xT.rearrange("(do p) t -> p do t", p=P)
    with tile.TileContext(nc) as tc:
        with (
            tc.tile_pool(name="sb", bufs=2) as sb,
            tc.tile_pool(name="consts", bufs=1) as cp,
            tc.tile_pool(name="xp", bufs=2) as xp,
            tc.tile_pool(name="ps", bufs=2, space="PSUM") as ps,
        ):
            gw_sb = cp.tile([P, DO, E], F32, tag="gw")
            nc.sync.dma_start(gw_sb[:], gw.rearrange("(do p) e -> p do e", p=P))
            for tt in range(T // NT):
                # per-(token-tile, d-tile) x chunks (256KB) so the first
                # matmul starts as soon as the first chunk lands
                xdi = []
                for di in range(DO):
                    xt = xp.tile([P, NT], F32, tag=f"x{di}", name=f"x{di}")
                    eng = nc.sync if di % 2 == 0 else nc.scalar
                    eng.dma_start(xt[:], xT_r[:, di, tt * NT:(tt + 1) * NT])
                    xdi.append(xt)
                pt = ps.tile([E, NT], F32, tag="pt")
                for di in range(DO):
                    nc.tensor.matmul(
                        pt[:],
                        gw_sb[:, di],
                        xdi[di][:],
                        start=(di == 0),
                        stop=(di == DO - 1),
                    )
                ot = sb.tile([E, NT], F32, tag="ot")
                nc.vector.tensor_copy(ot[:], pt[:])
                nc.sync.dma_start(logitsT[:, tt * NT:(tt + 1) * NT], ot[:])
    nc.finalize()
    return nc


def _build_expert(D, H, O, C):
    """Per-core expert FFN over C (padded) routed tokens.

    y[C, O] = (relu(x @ W1 + b1) @ W2 + b2) * w_token[:, None]
    computed as hT = W1.T-slices @ xT (keeps H on partitions), then
    y = hT-slices.T @ W2 (tokens back on partitions). No transposes on
    device: xT / b1 / wt come host-prearranged.
    """
    nc = bacc.Bacc(None, target_bir_lowering=False)
    xT = nc.dram_tensor("xT", [D, C], BF16, kind="ExternalInput")
    w1 = nc.dram_tensor("w1", [D, H], BF16, kind="ExternalInput")
    w2 = nc.dram_tensor("w2", [H, O], BF16, kind="ExternalInput")
    b1 = nc.dram_tensor("b1", [P, H // P], F32, kind="ExternalInput")
    b2 = nc.dram_tensor("b2", [P, O], F32, kind="ExternalInput")
    wt = nc.dram_tensor("wt", [P, C // P], F32, kind="ExternalInput")
    y = nc.dram_tensor("y", [C, O], F32, kind="ExternalOutput")
    DO, HO = D // P, H // P
    OO = O // 512
    # token blocks of CB, trailing 128-block if C % CB != 0
    starts = []
    pos = 0
    while pos < C:
        cb = CB if C - pos >= CB else P
        starts.append((pos, cb))
        pos += cb
    # chunk the weight loads so the first matmuls start after ~1MB of DMA;
    # the first two W1 chunks are half-size so L1 starts even earlier
    HC = 4                   # h-tiles (of 128) per W2 weight chunk
    NWC = HO // HC           # number of W2 weight chunks
    w1_chunks = [(0, 2), (2, 2)] + [(h, 4) for h in range(4, HO, 4)]
    w1_of_hi = {}            # hi -> (chunk index, offset within chunk)
    for ci, (h0, nh) in enumerate(w1_chunks):
        for j in range(nh):
            w1_of_hi[h0 + j] = (ci, j)
    HG = 8                   # h-tiles per hT group tile (finer L2 deps)
    NHG = HO // HG
    y_r = y.rearrange("(n p) o -> p n o", p=P)
    w1_r = w1.rearrange("(do p) h -> p do h", p=P)
    w2_r = w2.rearrange("(ho p) o -> p ho o", p=P)
    with tile.TileContext(nc) as tc:
        with (
            tc.tile_pool(name="wpool", bufs=1) as wp,
            tc.tile_pool(name="xpool", bufs=3) as xp,
            tc.tile_pool(name="hpool", bufs=2) as hp,
            tc.tile_pool(name="opool", bufs=4) as op,
            tc.tile_pool(name="hps", bufs=4, space="PSUM") as hps,
            tc.tile_pool(name="yps", bufs=3, space="PSUM") as yps,
        ):
            xT_r = xT.rearrange("(do p) c -> p do c", p=P)
            # startup-critical DMAs: W1 chunks in consumption order on the
            # sync ring (L1 of block 0 chases W1's delivery); block-0 x,
            # W2 chunk 0 and biases on the scalar HWDGE ring.
            x0_sb = xp.tile([P, DO, CB], BF16, tag="x")
            nc.scalar.dma_start(x0_sb[:, :, :starts[0][1]], xT_r[:, :, 0:starts[0][1]])
            w1c = [wp.tile([P, DO, nh * P], BF16, tag=f"w1_{k}", name=f"w1_{k}")
                   for k, (h0, nh) in enumerate(w1_chunks)]
            w2c = [wp.tile([P, HC, O], BF16, tag=f"w2_{k}", name=f"w2_{k}") for k in range(NWC)]
            for k, (h0, nh) in enumerate(w1_chunks):
                nc.sync.dma_start(w1c[k][:], w1_r[:, :, h0 * P:(h0 + nh) * P])
            b1_sb = wp.tile([P, HO], F32, tag="b1")
            nc.scalar.dma_start(b1_sb[:], b1[:])
            nc.scalar.dma_start(w2c[0][:], w2_r[:, 0:HC])
            b2_sb = wp.tile([P, O], F32, tag="b2")
            nc.scalar.dma_start(b2_sb[:], b2[:])
            wt_sb = wp.tile([P, C // P], F32, tag="wt")
            nc.scalar.dma_start(wt_sb[:], wt[:])

            # W2 chunks 1.. are paced behind block-0 relus so they don't
            # race the critical W1 stream during startup
            w2_load_after = {
                4 * k: [(w2c[k], w2_r[:, k * HC:(k + 1) * HC])]
                for k in range(1, NWC)
            }
            for blk, (n0, cb) in enumerate(starts):
                if blk == 0:
                    x_sb = x0_sb[:, :, :cb]
                else:
                    x_sb = xp.tile([P, DO, CB], BF16, tag="x", name="x_sb")[:, :, :cb]
                    nc.sync.dma_start(x_sb[:], xT_r[:, :, n0:n0 + cb])
                hgs = [hp.tile([P, HG, CB], BF16, tag=f"h{g}", name=f"h{g}")[:, :, :cb]
                       for g in range(NHG)]
                for hi in range(HO):
                    ph = hps.tile([P, CB], F32, tag="ph", name="ph")[:, :cb]
                    ci, off = w1_of_hi[hi]
                    for di in range(DO):
                        nc.tensor.matmul(
                            ph[:],
                            w1c[ci][:, di, off * P:(off + 1) * P],
                            x_sb[:, di],
                            start=(di == 0),
                            stop=(di == DO - 1),
                        )
                    act = nc.scalar.activation(
                        hgs[hi // HG][:, hi % HG], ph[:],
                        mybir.ActivationFunctionType.Relu,
                        bias=b1_sb[:, hi:hi + 1],
                    )
                    if blk == 0 and hi in w2_load_after:
                        # W2 chunk k streams only after L1 consumed W1 chunk
                        # k, so it never races the critical W1 delivery
                        for w2t, w2src in w2_load_after[hi]:
                            dma = nc.scalar.dma_start(w2t[:], w2src)
                            tile.add_dep_helper(
                                dma.ins, act.ins,
                                reason="pace late load behind W1 consumption",
                            )
                for ct in range(cb // P):
                    # hi outer / ot inner: both ot matmuls share the same
                    # stationary hT slice, halving LDWEIGHTS pressure
                    yps_ct = [yps.tile([P, 512], F32, tag="yp", name="yp")
                              for _ in range(OO)]
                    for hi in range(HO):
                        for ot in range(OO):
                            nc.tensor.matmul(
                                yps_ct[ot][:],
                                hgs[hi // HG][:, hi % HG, ct * P:(ct + 1) * P],
                                w2c[hi // HC][:, hi % HC, ot * 512:(ot + 1) * 512],
                                start=(hi == 0),
                                stop=(hi == HO - 1),
                            )
                    for ot in range(OO):
                        o_sb = op.tile([P, 512], F32, tag="o")
                        nc.vector.tensor_add(
                            o_sb[:], yps_ct[ot][:], b2_sb[:, ot * 512:(ot + 1) * 512]
                        )
                        n_idx = n0 // P + ct
                        nc.vector.tensor_scalar_mul(
                            o_sb[:], o_sb[:], wt_sb[:, n_idx:n_idx + 1]
                        )
                        nc.sync.dma_start(
                            y_r[:, n_idx, ot * 512:(ot + 1) * 512], o_sb[:]
                        )
    nc.finalize()
    return nc


def kernel(x, W1, b1, W2, b2, gate_w, gate_b):
    x = np.ascontiguousarray(x, dtype=np.float32)
    W1 = np.asarray(W1, dtype=np.float32)
    b1 = np.asarray(b1, dtype=np.float32)
    W2 = np.asarray(W2, dtype=np.float32)
    b2 = np.asarray(b2, dtype=np.float32)
    gate_w = np.ascontiguousarray(gate_w, dtype=np.float32)
    gate_b = np.asarray(gate_b, dtype=np.float32)

    B, D = x.shape
    E, _, H = W1.shape
    O = W2.shape[2]
    assert E == N_CORES and B % (N_CORES * 512) == 0 and D % P == 0
    T = B // N_CORES
    core_ids = list(range(N_CORES))

    # ---- Phase 1: gate logits on device (data-parallel over tokens) ----
    key = ("gate", D, E, T)
    if key not in _build_cache:
        _build_cache[key] = _build_gate(D, E, T)
    nc_gate = _build_cache[key]
    in_maps = [
        {"xT": np.ascontiguousarray(x[i * T:(i + 1) * T].T), "gw": gate_w}
        for i in range(N_CORES)
    ]
    res = run_bass_kernel_spmd(nc_gate, in_maps, core_ids=core_ids)
    logits = np.concatenate(
        [res.results[i]["logitsT"].T for i in range(N_CORES)], axis=0
    ) + gate_b[None, :]

    # ---- Host: top-2 routing (the expert-parallel sharding decision) ----
    lg = logits.astype(np.float64)
    lg -= lg.max(axis=1, keepdims=True)
    probs = np.exp(lg)
    probs /= probs.sum(axis=1, keepdims=True)
    order = np.argsort(-probs, axis=1, kind="stable")[:, :2]
    p_top = np.take_along_axis(probs, order, axis=1)
    w_top = p_top / p_top.sum(axis=1, keepdims=True)  # [B, 2]

    idx_e, wt_e = [], []
    for e in range(E):
        m0 = order[:, 0] == e
        m1 = order[:, 1] == e
        sel = m0 | m1
        idx = np.nonzero(sel)[0]
        w = np.where(m0[sel], w_top[sel, 0], w_top[sel, 1]).astype(np.float32)
        idx_e.append(idx)
        wt_e.append(w)
    max_count = max(len(i) for i in idx_e)
    C = max(CB, ((max_count + P - 1) // P) * P)

    # ---- Phase 2: expert FFN on device (expert-parallel) ----
    key = ("expert", D, H, O, C)
    if key not in _build_cache:
        _build_cache[key] = _build_expert(D, H, O, C)
    nc_exp = _build_cache[key]

    in_maps = []
    for e in range(E):
        n_e = len(idx_e[e])
        xT_pad = np.zeros((D, C), dtype=_bf16_np)
        xT_pad[:, :n_e] = x[idx_e[e]].T.astype(_bf16_np)
        wt_pad = np.zeros(C, dtype=np.float32)
        wt_pad[:n_e] = wt_e[e]
        in_maps.append({
            "xT": xT_pad,
            "w1": W1[e].astype(_bf16_np),
            "w2": W2[e].astype(_bf16_np),
            "b1": np.ascontiguousarray(b1[e].reshape(H // P, P).T),
            "b2": np.ascontiguousarray(np.broadcast_to(b2[e], (P, O))),
            "wt": np.ascontiguousarray(wt_pad.reshape(C // P, P).T),
        })
    res = run_bass_kernel_spmd(nc_exp, in_maps, core_ids=core_ids)

    # ---- Host: un-permute and combine the two expert contributions ----
    out = np.zeros((B, O), dtype=np.float32)
    for e in range(E):
        n_e = len(idx_e[e])
        if n_e:
            out[idx_e[e]] += res.results[e]["y"][:n_e]
    return out



# revision 5
# speedup vs baseline: 1.0826x; 1.0826x over previous
"""MoE layer (top-2 of 8 experts) on 8 TRN2 NeuronCores.

Strategy:
  Host: gate logits (fp64) + softmax + top-2 + renormalized weights — the
      routing / sharding decision. Build per-expert token lists, pad to a
      common capacity C (multiple of 128).
  Device (one launch, expert-parallel): core e runs its expert's FFN over
      the tokens routed to it: y = (relu(x@W1+b1)@W2 + b2) * w_token.
      bf16 matmuls, fp32 PSUM accumulation, weights SBUF-resident.
      L1 (x@W1) and L2 (h@W2) are interleaved per h-tile so the W1+W2
      weight streams spread over the whole first token block instead of
      bursting above HBM bandwidth.
  Host: scatter-add the two scaled contributions per token.
"""

import numpy as np
import ml_dtypes

import concourse.mybir as mybir
import concourse.tile as tile
from concourse import bacc
from concourse.bass_utils import run_bass_kernel_spmd

P = 128
N_CORES = 8
CB = 256  # token block
LAG = 2   # L2 trails L1 by this many h-tiles
BF16 = mybir.dt.bfloat16
F32 = mybir.dt.float32
_bf16_np = ml_dtypes.bfloat16

_build_cache = {}


def _build_expert(D, H, O, C, has_b2):
    """Per-core expert FFN over C (padded) routed tokens.

    y[C, O] = (relu(x @ W1 + b1) @ W2 [+ b2]) * w_token[:, None]
    computed as hT = W1.T-slices @ xT (H on partitions), then
    y = hT-slices.T @ W2 (tokens back on partitions). L1 and L2 are
    interleaved per h-tile (L2 lags L1 by LAG tiles) so each block's
    weight consumption is spread over the block's whole span.
    """
    nc = bacc.Bacc(None, target_bir_lowering=False)
    xT = nc.dram_tensor("xT", [D, C], BF16, kind="ExternalInput")
    w1 = nc.dram_tensor("w1", [D, H], BF16, kind="ExternalInput")
    w2 = nc.dram_tensor("w2", [H, O], BF16, kind="ExternalInput")
    b1 = nc.dram_tensor("b1", [P, H // P], F32, kind="ExternalInput")
    wt = nc.dram_tensor("wt", [P, C // P], F32, kind="ExternalInput")
    if has_b2:
        b2r = nc.dram_tensor("b2r", [1, O], F32, kind="ExternalInput")
    y = nc.dram_tensor("y", [C, O], F32, kind="ExternalOutput")
    DO, HO = D // P, H // P
    OO = O // 512
    # token blocks of CB, trailing 128-block if C % CB != 0
    starts = []
    pos = 0
    while pos < C:
        cb = CB if C - pos >= CB else P
        starts.append((pos, cb))
        pos += cb
    # W1 chunk schedule: first two chunks single-tile so L1 starts after
    # ~256KB of DMA; the rest 4-tile (1MB) chunks, issued in consumption
    # order on the sync ring.
    w1_chunks = [(0, 1), (1, 1), (2, 2), (4, 4)] + [(h, 4) for h in range(8, HO, 4)]
    w1_of_hi = {}
    for ci, (h0, nh) in enumerate(w1_chunks):
        for j in range(nh):
            w1_of_hi[h0 + j] = (ci, j)
    HC = 4                   # h-tiles per W2 chunk
    NWC = HO // HC
    y_r = y.rearrange("(n p) o -> p n o", p=P)
    w1_r = w1.rearrange("(do p) h -> p do h", p=P)
    w2_r = w2.rearrange("(ho p) o -> p ho o", p=P)
    xT_r = xT.rearrange("(do p) c -> p do c", p=P)
    with tile.TileContext(nc) as tc:
        with (
            tc.tile_pool(name="wpool", bufs=1) as wp,
            tc.tile_pool(name="xpool", bufs=3) as xp,
            tc.tile_pool(name="hpool", bufs=4) as hp,
            tc.tile_pool(name="opool", bufs=4) as op,
            tc.tile_pool(name="hps", bufs=3, space="PSUM") as hps,
            tc.tile_pool(name="yps", bufs=1, space="PSUM") as yps,
        ):
            # startup DMAs. sync ring: W1 chunks in consumption order.
            # scalar ring: block-0 x, consts, then paced W2 chunks.
            x0_sb = xp.tile([P, DO, CB], BF16, tag="x")
            nc.scalar.dma_start(x0_sb[:, :, :starts[0][1]], xT_r[:, :, 0:starts[0][1]])
            w1c = [wp.tile([P, DO, nh * P], BF16, tag=f"w1_{k}", name=f"w1_{k}")
                   for k, (h0, nh) in enumerate(w1_chunks)]
            for k, (h0, nh) in enumerate(w1_chunks):
                nc.sync.dma_start(w1c[k][:], w1_r[:, :, h0 * P:(h0 + nh) * P])
            b1_sb = wp.tile([P, HO], F32, tag="b1")
            nc.scalar.dma_start(b1_sb[:], b1[:])
            wt_sb = wp.tile([P, C // P], F32, tag="wt")
            nc.scalar.dma_start(wt_sb[:], wt[:])
            w2c = [wp.tile([P, HC, O], BF16, tag=f"w2_{k}", name=f"w2_{k}")
                   for k in range(NWC)]
            nc.scalar.dma_start(w2c[0][:], w2_r[:, 0:HC])
            if has_b2:
                b2_sb = wp.tile([1, O], F32, tag="b2")
                nc.scalar.dma_start(b2_sb[:], b2r[:])
                ones_sb = wp.tile([1, P], F32, tag="ones")
                nc.vector.memset(ones_sb[:], 1.0)

            # W2 chunk k is needed by the L2 of h-tile 4k (bracket 4k+LAG
            # of block 0); release its DMA once block-0 relu(4k-6) is done
            # so it never races the critical W1 stream.
            w2_pace = {max(0, 4 * k - 6): k for k in range(1, NWC)}

            for blk, (n0, cb) in enumerate(starts):
                if blk == 0:
                    x_sb = x0_sb[:, :, :cb]
                else:
                    x_sb = xp.tile([P, DO, CB], BF16, tag="x", name="x_sb")[:, :, :cb]
                    nc.gpsimd.dma_start(x_sb[:], xT_r[:, :, n0:n0 + cb])
                nct = cb // P
                # per-block PSUM accumulators, one bank per (ct, ot)
                yb = [[yps.tile([P, 512], F32, tag=f"y{ct}{ot}", name=f"y{ct}{ot}")
                       for ot in range(OO)] for ct in range(nct)]
                if has_b2:
                    for ct in range(nct):
                        for ot in range(OO):
                            nc.tensor.matmul(
                                yb[ct][ot][:],
                                ones_sb[:, 0:P],
                                b2_sb[:, ot * 512:(ot + 1) * 512],
                                start=True, stop=False,
                            )
                hts = {}

                def do_l1(hi):
                    ph = hps.tile([P, CB], F32, tag="ph", name="ph")[:, :cb]
                    ci, off = w1_of_hi[hi]
                    for di in range(DO):
                        nc.tensor.matmul(
                            ph[:],
                            w1c[ci][:, di, off * P:(off + 1) * P],
                            x_sb[:, di],
                            start=(di == 0),
                            stop=(di == DO - 1),
                        )
                    ht = hp.tile([P, CB], BF16, tag="h", name="h")[:, :cb]
                    act = nc.scalar.activation(
                        ht, ph[:],
                        mybir.ActivationFunctionType.Relu,
                        bias=b1_sb[:, hi:hi + 1],
                    )
                    hts[hi] = ht
                    if blk == 0 and hi in w2_pace:
                        k = w2_pace[hi]
                        dma = nc.scalar.dma_start(w2c[k][:], w2_r[:, k * HC:(k + 1) * HC])
                        tile.add_dep_helper(
                            dma.ins, act.ins,
                            reason="pace W2 chunk behind W1 consumption",
                        )

                def do_l2(hj):
                    ht = hts.pop(hj)
                    first = (hj == 0) and not has_b2
                    last = (hj == HO - 1)
                    for ct in range(nct):
                        for ot in range(OO):
                            nc.tensor.matmul(
                                yb[ct][ot][:],
                                ht[:, ct * P:(ct + 1) * P],
                                w2c[hj // HC][:, hj % HC, ot * 512:(ot + 1) * 512],
                                start=first,
                                stop=last,
                            )

                for hi in range(HO):
                    do_l1(hi)
                    if hi >= LAG:
                        do_l2(hi - LAG)
                for hj in range(HO - LAG, HO):
                    do_l2(hj)

                # drain: scale by the routing weight; split across the
                # scalar and vector engines so the banks free quickly.
                for ct in range(nct):
                    n_idx = n0 // P + ct
                    for ot in range(OO):
                        o_sb = op.tile([P, 512], F32, tag="o")
                        if (ct + ot) % 2 == 0:
                            nc.scalar.activation(
                                o_sb[:], yb[ct][ot][:],
                                mybir.ActivationFunctionType.Copy,
                                scale=wt_sb[:, n_idx:n_idx + 1],
                            )
                        else:
                            nc.vector.tensor_scalar_mul(
                                o_sb[:], yb[ct][ot][:], wt_sb[:, n_idx:n_idx + 1]
                            )
                        nc.sync.dma_start(
                            y_r[:, n_idx, ot * 512:(ot + 1) * 512], o_sb[:]
                        )
    nc.finalize()
    return nc


def kernel(x, W1, b1, W2, b2, gate_w, gate_b):
    x = np.ascontiguousarray(x, dtype=np.float32)
    W1 = np.asarray(W1, dtype=np.float32)
    b1 = np.asarray(b1, dtype=np.float32)
    W2 = np.asarray(W2, dtype=np.float32)
    b2 = np.asarray(b2, dtype=np.float32)
    gate_w = np.ascontiguousarray(gate_w, dtype=np.float32)
    gate_b = np.asarray(gate_b, dtype=np.float32)

    B, D = x.shape
    E, _, H = W1.shape
    O = W2.shape[2]
    assert E == N_CORES and B % N_CORES == 0 and D % P == 0
    core_ids = list(range(N_CORES))

    # ---- Host: gating + top-2 routing (the sharding decision) ----
    lg = x.astype(np.float64) @ gate_w.astype(np.float64) + gate_b.astype(np.float64)
    lg -= lg.max(axis=1, keepdims=True)
    probs = np.exp(lg)
    probs /= probs.sum(axis=1, keepdims=True)
    order = np.argsort(-probs, axis=1, kind="stable")[:, :2]
    p_top = np.take_along_axis(probs, order, axis=1)
    w_top = (p_top / p_top.sum(axis=1, keepdims=True)).astype(np.float32)  # [B, 2]

    idx_e, wt_e = [], []
    for e in range(E):
        m0 = order[:, 0] == e
        m1 = order[:, 1] == e
        sel = m0 | m1
        idx = np.nonzero(sel)[0]
        w = np.where(m0[sel], w_top[sel, 0], w_top[sel, 1]).astype(np.float32)
        idx_e.append(idx)
        wt_e.append(w)
    max_count = max(len(i) for i in idx_e)
    C = max(CB, ((max_count + P - 1) // P) * P)

    has_b2 = bool(np.any(b2))

    # ---- Device: expert FFN (expert-parallel) ----
    key = ("expert", D, H, O, C, has_b2)
    if key not in _build_cache:
        _build_cache[key] = _build_expert(D, H, O, C, has_b2)
    nc_exp = _build_cache[key]

    in_maps = []
    for e in range(E):
        n_e = len(idx_e[e])
        xT_pad = np.zeros((D, C), dtype=_bf16_np)
        xT_pad[:, :n_e] = x[idx_e[e]].T.astype(_bf16_np)
        wt_pad = np.zeros(C, dtype=np.float32)
        wt_pad[:n_e] = wt_e[e]
        m = {
            "xT": xT_pad,
            "w1": W1[e].astype(_bf16_np),
            "w2": W2[e].astype(_bf16_np),
            "b1": np.ascontiguousarray(b1[e].reshape(H // P, P).T),
            "wt": np.ascontiguousarray(wt_pad.reshape(C // P, P).T),
        }
        if has_b2:
            m["b2r"] = np.ascontiguousarray(b2[e].reshape(1, O))
        in_maps.append(m)
    res = run_bass_kernel_spmd(nc_exp, in_maps, core_ids=core_ids)

    # ---- Host: un-permute and combine the two expert contributions ----
    out = np.zeros((B, O), dtype=np.float32)
    for e in range(E):
        n_e = len(idx_e[e])
        if n_e:
            out[idx_e[e]] += res.results[e]["y"][:n_e]
    return out


# revision 8
# speedup vs baseline: 1.1000x; 1.0161x over previous
"""MoE layer (top-2 of 8 experts) on 8 TRN2 NeuronCores.

Strategy (paired-expert tensor-parallel):
  Host: gate logits (fp64) + softmax + top-2 + renormalized weights — the
      routing / sharding decision. Experts are paired heavy+light by token
      count; cores 2i and 2i+1 each process the PAIR's full token list but
      only half of the hidden dim H (column-split W1, row-split W2), so
      every core does ~(B*K/E) tokens of half-H work — near-perfect load
      balance. Partial outputs of the two halves are summed on the host.
  Device (one launch): per block of 256 tokens, L1 (x@W1-half) and
      L2 (h@W2-half) are interleaved per h-tile so weight streaming is
      spread across each block's whole span. bf16 matmuls, fp32 PSUM.
      The drain scales by the routing weight (free: it rides the
      PSUM->SBUF copy).
  Host: sum half-contributions, scatter-add per token, add b2 if nonzero.
"""

import numpy as np
import ml_dtypes

import concourse.mybir as mybir
import concourse.tile as tile
from concourse import bacc
from concourse.bass_utils import run_bass_kernel_spmd

P = 128
N_CORES = 8
CB = 256  # token block
LAG = 2   # L2 trails L1 by this many h-tiles
BF16 = mybir.dt.bfloat16
F32 = mybir.dt.float32
_bf16_np = ml_dtypes.bfloat16

_build_cache = {}


def _block_list(c0, c1):
    out = []
    pos = c0
    while pos < c1:
        cb = CB if c1 - pos >= CB else P
        out.append((pos, cb))
        pos += cb
    return out


def _build_pair(D, H2, O, Ca, Cb):
    """One core's half-H FFN over two experts' (padded) routed tokens.

    Tokens [0, Ca) use expert-set 0's weights, [Ca, Ca+Cb) expert-set 1's.
    y[C, O] = relu(x @ W1half + b1half) @ W2half * w_token[:, None]
    (partial over H — the other core of the pair holds the other half).
    """
    C = Ca + Cb
    nc = bacc.Bacc(None, target_bir_lowering=False)
    xT = nc.dram_tensor("xT", [D, C], BF16, kind="ExternalInput")
    w1s = [nc.dram_tensor(f"w1{s}", [D, H2], BF16, kind="ExternalInput") for s in range(2)]
    w2s = [nc.dram_tensor(f"w2{s}", [H2, O], BF16, kind="ExternalInput") for s in range(2)]
    b1s = [nc.dram_tensor(f"b1{s}", [P, H2 // P], F32, kind="ExternalInput") for s in range(2)]
    wt = nc.dram_tensor("wt", [P, C // P], F32, kind="ExternalInput")
    y = nc.dram_tensor("y", [C, O], F32, kind="ExternalOutput")
    DO, HO = D // P, H2 // P
    OO = O // 512
    HC = 2                   # h-tiles per W2 chunk
    NWC = HO // HC
    blocks = [(n0, cb, 0) for n0, cb in _block_list(0, Ca)] + \
             [(n0, cb, 1) for n0, cb in _block_list(Ca, C)]
    nA = sum(1 for b in blocks if b[2] == 0)
    # W1 chunk schedules (in h-tile units). Segment 0 leads with
    # single-tile chunks so the first matmul starts after ~256KB.
    w1_chunks = [
        [(0, 1), (1, 1), (2, 2)] + [(h, 2) for h in range(4, HO, 2)],
        [(h, 2) for h in range(0, HO, 2)],
    ]
    w1_of_hi = []
    for s in range(2):
        m = {}
        for ci, (h0, nh) in enumerate(w1_chunks[s]):
            for j in range(nh):
                m[h0 + j] = (ci, j)
        w1_of_hi.append(m)
    y_r = y.rearrange("(n p) o -> p n o", p=P)
    w1_r = [w.rearrange("(do p) h -> p do h", p=P) for w in w1s]
    w2_r = [w.rearrange("(ho p) o -> p ho o", p=P) for w in w2s]
    xT_r = xT.rearrange("(do p) c -> p do c", p=P)
    with tile.TileContext(nc) as tc:
        with (
            tc.tile_pool(name="wpool", bufs=1) as wp,
            tc.tile_pool(name="xpool", bufs=3) as xp,
            tc.tile_pool(name="hpool", bufs=4) as hp,
            tc.tile_pool(name="opool", bufs=4) as op,
            tc.tile_pool(name="hps", bufs=3, space="PSUM") as hps,
            tc.tile_pool(name="yps", bufs=1, space="PSUM") as yps,
        ):
            # --- startup DMAs ---
            # block-0 x on the vector ring; segment-0 W1 chunks alternate
            # sync/gpsimd so early delivery isn't one ring's bandwidth;
            # consts + first W2 chunk on scalar.
            x0_sb = xp.tile([P, DO, CB], BF16, tag="x")
            nc.scalar.dma_start(x0_sb[:, :, :blocks[0][1]], xT_r[:, :, 0:blocks[0][1]])
            w1t = [[wp.tile([P, DO, nh * P], BF16, tag=f"w1_{s}_{k}", name=f"w1_{s}_{k}")
                    for k, (h0, nh) in enumerate(w1_chunks[s])] for s in range(2)]
            for k, (h0, nh) in enumerate(w1_chunks[0]):
                eng = nc.sync if k % 2 == 0 else nc.gpsimd
                eng.dma_start(w1t[0][k][:], w1_r[0][:, :, h0 * P:(h0 + nh) * P])
            b1t = []
            for s in range(2):
                t = wp.tile([P, HO], F32, tag=f"b1_{s}")
                nc.scalar.dma_start(t[:], b1s[s][:])
                b1t.append(t)
            wt_sb = wp.tile([P, C // P], F32, tag="wt")
            nc.scalar.dma_start(wt_sb[:], wt[:])
            w2t = [[wp.tile([P, HC, O], BF16, tag=f"w2_{s}_{k}", name=f"w2_{s}_{k}")
                    for k in range(NWC)] for s in range(2)]
            nc.scalar.dma_start(w2t[0][0][:], w2_r[0][:, 0:HC])

            # --- paced weight deliveries: {(blk, hi): [(engine, tile, src)]} ---
            paced = {}
            # W2 seg-0 chunks 1.. trickle behind block-0 relu progress.
            for k in range(1, NWC):
                paced.setdefault((0, max(0, 2 * k - 3)), []).append(
                    (nc.scalar, w2t[0][k], w2_r[0][:, k * HC:(k + 1) * HC]))
            # Segment-1 weights stream during segment 0's later blocks.
            if nA >= 6:
                for k, (h0, nh) in enumerate(w1_chunks[1]):
                    blk = 1 + k // 2
                    paced.setdefault((blk, (k % 2) * (HO // 2)), []).append(
                        (nc.gpsimd, w1t[1][k], w1_r[1][:, :, h0 * P:(h0 + nh) * P]))
                for k in range(NWC):
                    blk = min(nA - 1, 5 + k // 2)
                    paced.setdefault((blk, (k % 2) * (HO // 2)), []).append(
                        (nc.scalar, w2t[1][k], w2_r[1][:, k * HC:(k + 1) * HC]))
            else:
                for k, (h0, nh) in enumerate(w1_chunks[1]):
                    nc.gpsimd.dma_start(w1t[1][k][:], w1_r[1][:, :, h0 * P:(h0 + nh) * P])
                for k in range(NWC):
                    nc.scalar.dma_start(w2t[1][k][:], w2_r[1][:, k * HC:(k + 1) * HC])

            ndma = [0]
            for bi, (n0, cb, seg) in enumerate(blocks):
                if bi == 0:
                    x_sb = x0_sb[:, :, :cb]
                else:
                    x_sb = xp.tile([P, DO, CB], BF16, tag="x", name="x_sb")[:, :, :cb]
                    nc.gpsimd.dma_start(x_sb[:], xT_r[:, :, n0:n0 + cb])
                nct = cb // P
                yb = [[yps.tile([P, 512], F32, tag=f"y{ct}{ot}", name=f"y{ct}{ot}")
                       for ot in range(OO)] for ct in range(nct)]
                hts = {}

                def do_l1(hi, bi=bi, seg=seg, x_sb=x_sb, hts=hts, cb=cb):
                    ph = hps.tile([P, CB], F32, tag="ph", name="ph")[:, :cb]
                    ci, off = w1_of_hi[seg][hi]
                    for di in range(DO):
                        nc.tensor.matmul(
                            ph[:],
                            w1t[seg][ci][:, di, off * P:(off + 1) * P],
                            x_sb[:, di],
                            start=(di == 0),
                            stop=(di == DO - 1),
                        )
                    ht = hp.tile([P, CB], BF16, tag="h", name="h")[:, :cb]
                    act = nc.scalar.activation(
                        ht, ph[:],
                        mybir.ActivationFunctionType.Relu,
                        bias=b1t[seg][:, hi:hi + 1],
                    )
                    hts[hi] = ht
                    for eng, wtile, src in paced.pop((bi, hi), []):
                        dma = eng.dma_start(wtile[:], src)
                        tile.add_dep_helper(
                            dma.ins, act.ins,
                            reason="pace weight stream behind compute",
                        )

                def do_l2(hj, seg=seg, hts=hts, yb=yb, nct=nct):
                    ht = hts.pop(hj)
                    for ct in range(nct):
                        for ot in range(OO):
                            nc.tensor.matmul(
                                yb[ct][ot][:],
                                ht[:, ct * P:(ct + 1) * P],
                                w2t[seg][hj // HC][:, hj % HC, ot * 512:(ot + 1) * 512],
                                start=(hj == 0),
                                stop=(hj == HO - 1),
                            )

                for hi in range(HO):
                    do_l1(hi)
                    if hi >= LAG:
                        do_l2(hi - LAG)
                for hj in range(HO - LAG, HO):
                    do_l2(hj)

                # drain: scale by routing weight during the PSUM->SBUF copy,
                # split across scalar/vector; y DMAs alternate sync/gpsimd.
                for ct in range(nct):
                    n_idx = n0 // P + ct
                    for ot in range(OO):
                        o_sb = op.tile([P, 512], F32, tag="o")
                        if (ct + ot) % 2 == 0:
                            nc.scalar.activation(
                                o_sb[:], yb[ct][ot][:],
                                mybir.ActivationFunctionType.Copy,
                                scale=wt_sb[:, n_idx:n_idx + 1],
                            )
                        else:
                            nc.vector.tensor_scalar_mul(
                                o_sb[:], yb[ct][ot][:], wt_sb[:, n_idx:n_idx + 1]
                            )
                        eng = nc.sync if ndma[0] % 2 == 0 else nc.gpsimd
                        ndma[0] += 1
                        eng.dma_start(
                            y_r[:, n_idx, ot * 512:(ot + 1) * 512], o_sb[:]
                        )
    nc.finalize()
    return nc


def kernel(x, W1, b1, W2, b2, gate_w, gate_b):
    x = np.ascontiguousarray(x, dtype=np.float32)
    W1 = np.asarray(W1, dtype=np.float32)
    b1 = np.asarray(b1, dtype=np.float32)
    W2 = np.asarray(W2, dtype=np.float32)
    b2 = np.asarray(b2, dtype=np.float32)
    gate_w = np.ascontiguousarray(gate_w, dtype=np.float32)
    gate_b = np.asarray(gate_b, dtype=np.float32)

    B, D = x.shape
    E, _, H = W1.shape
    O = W2.shape[2]
    assert E == N_CORES and B % N_CORES == 0 and D % P == 0
    H2 = H // 2
    assert H2 % P == 0 and O % 512 == 0
    core_ids = list(range(N_CORES))

    # ---- Host: gating + top-2 routing (the sharding decision) ----
    lg = x.astype(np.float64) @ gate_w.astype(np.float64) + gate_b.astype(np.float64)
    lg -= lg.max(axis=1, keepdims=True)
    probs = np.exp(lg)
    probs /= probs.sum(axis=1, keepdims=True)
    order = np.argsort(-probs, axis=1, kind="stable")[:, :2]
    p_top = np.take_along_axis(probs, order, axis=1)
    w_top = (p_top / p_top.sum(axis=1, keepdims=True)).astype(np.float32)  # [B, 2]

    idx_e, wt_e = [], []
    for e in range(E):
        m0 = order[:, 0] == e
        m1 = order[:, 1] == e
        sel = m0 | m1
        idx = np.nonzero(sel)[0]
        w = np.where(m0[sel], w_top[sel, 0], w_top[sel, 1]).astype(np.float32)
        idx_e.append(idx)
        wt_e.append(w)

    # pair heavy-with-light by routed token count for near-equal pair sums
    srt = sorted(range(E), key=lambda e: -len(idx_e[e]))
    pairs = [(srt[i], srt[E - 1 - i]) for i in range(E // 2)]
    Ca = max(CB, ((max(len(idx_e[a]) for a, _ in pairs) + P - 1) // P) * P)
    Cb = max(P, ((max(len(idx_e[b]) for _, b in pairs) + P - 1) // P) * P)
    C = Ca + Cb

    # ---- Device: paired-expert half-H FFN ----
    key = ("pair", D, H2, O, Ca, Cb)
    if key not in _build_cache:
        _build_cache[key] = _build_pair(D, H2, O, Ca, Cb)
    nc_exp = _build_cache[key]

    in_maps = []
    for i, (a, b) in enumerate(pairs):
        n_a, n_b = len(idx_e[a]), len(idx_e[b])
        xT_pad = np.zeros((D, C), dtype=_bf16_np)
        xT_pad[:, :n_a] = x[idx_e[a]].T.astype(_bf16_np)
        xT_pad[:, Ca:Ca + n_b] = x[idx_e[b]].T.astype(_bf16_np)
        wt_pad = np.zeros(C, dtype=np.float32)
        wt_pad[:n_a] = wt_e[a]
        wt_pad[Ca:Ca + n_b] = wt_e[b]
        wt_m = np.ascontiguousarray(wt_pad.reshape(C // P, P).T)
        for half in range(2):
            sl = slice(half * H2, (half + 1) * H2)
            in_maps.append({
                "xT": xT_pad,
                "w10": np.ascontiguousarray(W1[a][:, sl]).astype(_bf16_np),
                "w11": np.ascontiguousarray(W1[b][:, sl]).astype(_bf16_np),
                "w20": np.ascontiguousarray(W2[a][sl, :]).astype(_bf16_np),
                "w21": np.ascontiguousarray(W2[b][sl, :]).astype(_bf16_np),
                "b10": np.ascontiguousarray(b1[a][sl].reshape(H2 // P, P).T),
                "b11": np.ascontiguousarray(b1[b][sl].reshape(H2 // P, P).T),
                "wt": wt_m,
            })
    res = run_bass_kernel_spmd(nc_exp, in_maps, core_ids=core_ids)

    # ---- Host: sum the two half-H contributions, un-permute, combine ----
    out = np.zeros((B, O), dtype=np.float32)
    for i, (a, b) in enumerate(pairs):
        n_a, n_b = len(idx_e[a]), len(idx_e[b])
        yp = res.results[2 * i]["y"] + res.results[2 * i + 1]["y"]
        if n_a:
            out[idx_e[a]] += yp[:n_a]
        if n_b:
            out[idx_e[b]] += yp[Ca:Ca + n_b]
    if np.any(b2):
        out += w_top[:, 0, None] * b2[order[:, 0]] + w_top[:, 1, None] * b2[order[:, 1]]
    return out
